# revision 1
# baseline (speedup 1.0000x reference)
import numpy as np

# Per-head sparse MoE (top-2 of 8 experts), expert-parallel across 8 NeuronCores.
# B=8192 tokens, N=16 heads, D=128, H=384, E=8.
# Host: router (replicates reference jnp ops bit-exactly) + token dispatch.
# Device (core e = expert e): per head, h1=w1.T@x, g=wg.T@x, h=h1*silu(g),
# y=w2.T@h on capacity-padded token batches; the per-token routing weight is
# applied on the HOST during unpack (it commutes through the w2 contraction
# along the token axis), so the device never touches it.
# Engine assignment from microbenchmarks: silu 1024-wide on ACT (657ns/op,
# 2x cheaper per elem than 512); h1*sil muls 512-wide on DVE (430ns; 1024-wide
# PSUM-source DVE is pathological at 1608ns); PSUM->SBUF output evacuation
# split 2:3 between DVE (523ns) and ACT (681ns) to balance engine load.
# x/weights/y travel bf16; accumulate f32 in PSUM.

B, N, D, H, E = 8192, 16, 128, 384, 8
N_CORES = 8
HB = H // 128   # h-blocks of 128
WCOL = 3 * H    # packed weight columns per head: [w1 | wg | w2r]

USE_BF16 = True
PSUM_LAYOUT = "g2h2o2"   # or "g2h3o1"

_nc_cache = {}


def _build_bass(C, chunks=None, reps=1, bf16=USE_BF16, layout=None,
                dve_res=(0, 2), sp_bufs=2, hp_bufs=2, pipe=False):
    import concourse.bacc as bacc
    import concourse.mybir as mybir
    import concourse.tile as tile

    layout = layout or PSUM_LAYOUT
    f32 = mybir.dt.float32
    f32r = mybir.dt.float32r
    bf = mybir.dt.bfloat16
    din = bf if bf16 else f32
    dmm = bf if bf16 else f32r
    SILU = mybir.ActivationFunctionType.Silu

    # layout: gW = g tile width, bg/bh/bo = PSUM ring depths; h tiles are
    # [128,512] (1 bank) unless layout starts with a capital H (then 1024).
    cfg = {
        "g2h2o2": (1024, 2, 2, 2, 512),
        "g2h3o1": (1024, 2, 3, 1, 512),
        "g1H2o2": (1024, 1, 2, 2, 1024),
        "G1h2o2": (2048, 1, 2, 2, 512),
    }
    gW, bg, bh, bo, hW = cfg[layout]

    pairs = [(p0, min(gW, C - p0)) for p0 in range(0, C, gW)]

    def subs(pw):
        return [(s, min(512, pw - s)) for s in range(0, pw, 512)]

    nc = bacc.Bacc("TRN2", target_bir_lowering=False, debug=False, num_devices=N_CORES)
    xT = nc.dram_tensor("xT", [N, 128, C], din, kind="ExternalInput").ap()
    wall = nc.dram_tensor("wall", [N, 128, WCOL], din, kind="ExternalInput").ap()
    yT = nc.dram_tensor("yT", [N, 128, C], din, kind="ExternalOutput").ap()

    def cast(ap):
        return ap if bf16 else ap.bitcast(f32r)

    copy_cnt = [0]

    with tile.TileContext(nc) as tc:
        with tc.tile_pool(name="xp", bufs=2) as xp, \
             tc.tile_pool(name="wp", bufs=2) as wp, \
             tc.tile_pool(name="sp", bufs=sp_bufs) as sp, \
             tc.tile_pool(name="hp", bufs=hp_bufs) as hp, \
             tc.tile_pool(name="ob", bufs=4) as ob, \
             tc.tile_pool(name="pg", bufs=bg, space="PSUM") as pg, \
             tc.tile_pool(name="ph", bufs=bh, space="PSUM") as ph, \
             tc.tile_pool(name="po", bufs=bo, space="PSUM") as po:

            pending = [None]

            def emit_o(pend):
                n, p0, pw, hbts, w_t = pend
                w2of = 2 * H
                for (s0, sw) in subs(pw):
                    o_p = po.tile([128, 512], f32, tag="o")
                    for hb in range(HB):
                        nc.tensor.matmul(
                            o_p[:, :sw],
                            w_t[:, w2of + hb * 128:w2of + (hb + 1) * 128],
                            hbts[hb][:, s0:s0 + sw],
                            start=(hb == 0), stop=(hb == HB - 1),
                        )
                    o_sb = ob.tile([128, 512], din, tag="osb")
                    if copy_cnt[0] % 5 in dve_res:
                        nc.vector.tensor_copy(o_sb[:, :sw], o_p[:, :sw])
                    else:
                        nc.scalar.copy(o_sb[:, :sw], o_p[:, :sw])
                    copy_cnt[0] += 1
                    nc.scalar.dma_start(
                        yT[n][:, p0 + s0:p0 + s0 + sw], o_sb[:, :sw]
                    )

            def head(n):
                x_t = xp.tile([128, C], dmm, tag="x")
                nc.sync.dma_start(x_t[:], cast(xT[n]))
                w_t = wp.tile([128, WCOL], dmm, tag="w")
                nc.sync.dma_start(w_t[:], cast(wall[n]))
                w1of, wgof, w2of = 0, H, 2 * H
                for (p0, pw) in pairs:
                    hbts = []
                    for hb in range(HB):
                        g_t = pg.tile([128, gW], f32, tag="g")
                        for (s0, sw) in subs(pw):
                            nc.tensor.matmul(
                                g_t[:, s0:s0 + sw],
                                w_t[:, wgof + hb * 128:wgof + (hb + 1) * 128],
                                x_t[:, p0 + s0:p0 + s0 + sw],
                                start=True, stop=True,
                            )
                        sil = sp.tile([128, gW], f32, tag="sil")
                        nc.scalar.activation(sil[:, :pw], g_t[:, :pw], SILU)
                        hbt = hp.tile([128, gW], dmm, tag=f"hbt{hb}")
                        if hW == 512:
                            for (s0, sw) in subs(pw):
                                h1_t = ph.tile([128, 512], f32, tag="h1")
                                nc.tensor.matmul(
                                    h1_t[:, :sw],
                                    w_t[:, w1of + hb * 128:w1of + (hb + 1) * 128],
                                    x_t[:, p0 + s0:p0 + s0 + sw],
                                    start=True, stop=True,
                                )
                                nc.vector.tensor_mul(
                                    hbt[:, s0:s0 + sw], h1_t[:, :sw],
                                    sil[:, s0:s0 + sw]
                                )
                        else:
                            h1_t = ph.tile([128, hW], f32, tag="h1")
                            for (s0, sw) in subs(pw):
                                nc.tensor.matmul(
                                    h1_t[:, s0:s0 + sw],
                                    w_t[:, w1of + hb * 128:w1of + (hb + 1) * 128],
                                    x_t[:, p0 + s0:p0 + s0 + sw],
                                    start=True, stop=True,
                                )
                            for (s0, sw) in subs(pw):
                                nc.vector.tensor_mul(
                                    hbt[:, s0:s0 + sw], h1_t[:, s0:s0 + sw],
                                    sil[:, s0:s0 + sw]
                                )
                        hbts.append(hbt)
                        if pipe and hb == 0 and pending[0] is not None:
                            emit_o(pending[0])
                            pending[0] = None
                    if pipe:
                        pending[0] = (n, p0, pw, hbts, w_t)
                    else:
                        emit_o((n, p0, pw, hbts, w_t))

            def body():
                for n in range(N):
                    head(n)
                if pipe and pending[0] is not None:
                    emit_o(pending[0])
                    pending[0] = None

            if reps == 1:
                body()
            else:
                with tc.For_i(0, reps, 1):
                    body()
    nc.finalize()
    return nc


def _route(x, router_w):
    import jax
    import jax.numpy as jnp

    router_logits = jnp.asarray(x).reshape(B, N * D) @ jnp.asarray(router_w).T
    topk_logits, topk_idx = jax.lax.top_k(router_logits, 2)
    topk_w = jax.nn.softmax(topk_logits, axis=-1)
    return np.asarray(topk_idx), np.asarray(topk_w).astype(np.float32)


def _dispatch(x, topk_idx, topk_w):
    idx_list, wgt_list = [], []
    for e in range(E):
        sel = np.nonzero((topk_idx == e).any(axis=1))[0]
        we = np.where(topk_idx[sel, 0] == e, topk_w[sel, 0], topk_w[sel, 1])
        idx_list.append(sel)
        wgt_list.append(we.astype(np.float32))
    maxL = max(max(len(s) for s in idx_list), 1)
    C = ((maxL + 127) // 128) * 128
    chunks = []
    c0 = 0
    while c0 < C:
        cw = 512 if C - c0 >= 512 else C - c0
        chunks.append((c0, cw))
        c0 += cw
    return idx_list, wgt_list, C, tuple(chunks)


def _make_in_maps(x, w1, w_gate, w2, idx_list, wgt_list, C, bf16=USE_BF16):
    if bf16:
        import ml_dtypes
        dt = ml_dtypes.bfloat16
    else:
        dt = np.float32
    in_maps = []
    xTfull = np.ascontiguousarray(x.transpose(1, 2, 0).astype(dt))  # (N,128,B)
    for e in range(E):
        sel = idx_list[e]
        L = len(sel)
        xg = np.zeros((N, 128, C), dt)
        if L:
            xg[:, :, :L] = xTfull[:, :, sel]
        w2r = w2[e].reshape(N, HB, 128, 128).transpose(0, 2, 1, 3).reshape(N, 128, H)
        wcat = np.ascontiguousarray(np.concatenate(
            [w1[e].astype(dt), w_gate[e].astype(dt), w2r.astype(dt)], axis=2
        ))  # (N,128,3H)
        in_maps.append({"xT": xg, "wall": wcat})
    return in_maps


_runner_cache = {}


def _make_runner(nc):
    """Cached jitted executor equivalent to bass2jax.run_bass_via_pjrt,
    avoiding per-call retrace/rejit of the shard_map wrapper."""
    import jax
    import concourse.mybir as mybir
    from concourse import bass2jax
    from jax.sharding import Mesh, PartitionSpec
    from jax.experimental.shard_map import shard_map

    bass2jax.install_neuronx_cc_hook()
    partition_name = nc.partition_id_tensor.name if nc.partition_id_tensor else None
    in_names, out_names, out_avals, out_shapes = [], [], [], []
    for alloc in nc.m.functions[0].allocations:
        if not isinstance(alloc, mybir.MemoryLocationSet):
            continue
        name = alloc.memorylocations[0].name
        if alloc.kind == "ExternalInput":
            if name != partition_name:
                in_names.append(name)
        elif alloc.kind == "ExternalOutput":
            shape = tuple(alloc.tensor_shape)
            dtype = mybir.dt.np(alloc.dtype)
            out_names.append(name)
            out_avals.append(jax.core.ShapedArray(shape, dtype))
            out_shapes.append((shape, dtype))
    all_in_names = list(in_names) + list(out_names)
    if partition_name is not None:
        all_in_names.append(partition_name)

    def _body(*args):
        operands = list(args)
        if partition_name is not None:
            operands.append(bass2jax.partition_id_tensor())
        return tuple(bass2jax._bass_exec_p.bind(
            *operands,
            out_avals=tuple(out_avals),
            in_names=tuple(all_in_names),
            out_names=tuple(out_names),
            lowering_input_output_aliases=(),
            sim_require_finite=True,
            sim_require_nnan=True,
            nc=nc,
        ))

    mesh = Mesh(np.asarray(jax.devices()[:N_CORES]), ("core",))
    nio = len(in_names) + len(out_names)
    sharded = jax.jit(
        shard_map(_body, mesh=mesh,
                  in_specs=(PartitionSpec("core"),) * nio,
                  out_specs=(PartitionSpec("core"),) * len(out_names),
                  check_rep=False),
        keep_unused=True,
    )

    def run(in_maps):
        concat_in = [
            np.concatenate([np.asarray(in_maps[c][nm]) for c in range(N_CORES)],
                           axis=0)
            for nm in in_names
        ]
        concat_zeros = [
            np.zeros((N_CORES * s[0], *s[1:]), d) for (s, d) in out_shapes
        ]
        outs = sharded(*(concat_in + concat_zeros))
        outs = [np.asarray(o) for o in outs]
        results = []
        for c in range(N_CORES):
            res = {}
            for (nm, o, (s, d)) in zip(out_names, outs, out_shapes):
                res[nm] = o[c * s[0]:(c + 1) * s[0]]
            results.append(res)
        return results

    return run


def kernel(**inputs):
    x = np.asarray(inputs["x"], dtype=np.float32)
    router_w = np.asarray(inputs["router_w"], dtype=np.float32)
    w1 = np.asarray(inputs["w1"], dtype=np.float32)
    w_gate = np.asarray(inputs["w_gate"], dtype=np.float32)
    w2 = np.asarray(inputs["w2"], dtype=np.float32)

    topk_idx, topk_w = _route(x, router_w)
    idx_list, wgt_list, C, chunks = _dispatch(x, topk_idx, topk_w)

    key = (C, chunks, 1, USE_BF16)
    if key not in _nc_cache:
        _nc_cache[key] = _build_bass(C, chunks)
    nc = _nc_cache[key]

    in_maps = _make_in_maps(x, w1, w_gate, w2, idx_list, wgt_list, C)

    if key not in _runner_cache:
        from concourse import bass_utils
        res = bass_utils.run_bass_kernel_spmd(
            nc, in_maps, core_ids=list(range(N_CORES)), trace=False
        )
        results = res.results
        _runner_cache[key] = _make_runner(nc)
    else:
        results = _runner_cache[key](in_maps)

    out = np.zeros((B, N, D), np.float32)
    for e in range(E):
        sel = idx_list[e]
        L = len(sel)
        if L:
            yT = np.asarray(results[e]["yT"], dtype=np.float32)  # (N,128,C)
            out[sel] += yT[:, :, :L].transpose(2, 0, 1) * \
                wgt_list[e][:, None, None]
    return out



# revision 9
# speedup vs baseline: 1.1360x; 1.1360x over previous
import numpy as np

# Per-head sparse MoE (top-2 of 8 experts), expert-parallel across 8 NeuronCores.
# B=8192 tokens, N=16 heads, D=128, H=384, E=8.
# Host: router (replicates reference jnp ops bit-exactly) + token dispatch.
# Device (core e = expert e): per head, h1=w1.T@x, g=wg.T@x, h=h1*silu(g),
# y=w2.T@h on capacity-padded token batches; the per-token routing weight is
# applied on the HOST during unpack (it commutes through the w2 contraction
# along the token axis), so the device never touches it.
# Engine assignment from microbenchmarks: silu 1024-wide on ACT (657ns/op,
# 2x cheaper per elem than 512); h1*sil muls 512-wide on DVE (430ns; 1024-wide
# PSUM-source DVE is pathological at 1608ns); PSUM->SBUF output evacuation
# split 2:3 between DVE (523ns) and ACT (681ns) to balance engine load.
# x/weights/y travel bf16; accumulate f32 in PSUM.

B, N, D, H, E = 8192, 16, 128, 384, 8
N_CORES = 8
HB = H // 128   # h-blocks of 128
WCOL = 3 * H    # packed weight columns per head: [w1 | wg | w2r]

USE_BF16 = True
PSUM_LAYOUT = "g2h3o1"   # g ring 2 (2 banks each), h1 ring 3, o ring 1

_nc_cache = {}


def _build_bass(C, chunks=None, reps=1, bf16=USE_BF16, layout=None,
                dve_res=None, sp_bufs=6, hp_bufs=6, pipe=True,
                act_copy=(0, 1, 2), copy_mod=4, out_dma="sync"):
    import concourse.bacc as bacc
    import concourse.mybir as mybir
    import concourse.tile as tile

    layout = layout or PSUM_LAYOUT
    f32 = mybir.dt.float32
    f32r = mybir.dt.float32r
    bf = mybir.dt.bfloat16
    din = bf if bf16 else f32
    dmm = bf if bf16 else f32r
    SILU = mybir.ActivationFunctionType.Silu

    # layout: gW = g tile width, bg/bh/bo = PSUM ring depths; h tiles are
    # [128,512] (1 bank) unless layout starts with a capital H (then 1024).
    cfg = {
        "g2h2o2": (1024, 2, 2, 2, 512),
        "g2h3o1": (1024, 2, 3, 1, 512),
        "g1H2o2": (1024, 1, 2, 2, 1024),
        "G1h2o2": (2048, 1, 2, 2, 512),
    }
    gW, bg, bh, bo, hW = cfg[layout]

    pairs = [(p0, min(gW, C - p0)) for p0 in range(0, C, gW)]

    def subs(pw):
        return [(s, min(512, pw - s)) for s in range(0, pw, 512)]

    nc = bacc.Bacc("TRN2", target_bir_lowering=False, debug=False, num_devices=N_CORES)
    xT = nc.dram_tensor("xT", [N, 128, C], din, kind="ExternalInput").ap()
    wall = nc.dram_tensor("wall", [N, 128, WCOL], din, kind="ExternalInput").ap()
    yT = nc.dram_tensor("yT", [N, 128, C], din, kind="ExternalOutput").ap()

    def cast(ap):
        return ap if bf16 else ap.bitcast(f32r)

    copy_cnt = [0]

    with tile.TileContext(nc) as tc:
        with tc.tile_pool(name="xp", bufs=3) as xp, \
             tc.tile_pool(name="wp", bufs=3) as wp, \
             tc.tile_pool(name="sp", bufs=sp_bufs) as sp, \
             tc.tile_pool(name="hp", bufs=hp_bufs) as hp, \
             tc.tile_pool(name="ob", bufs=3) as ob, \
             tc.tile_pool(name="pg", bufs=bg, space="PSUM") as pg, \
             tc.tile_pool(name="ph", bufs=bh, space="PSUM") as ph, \
             tc.tile_pool(name="po", bufs=bo, space="PSUM") as po:

            pending = [None]

            def emit_o(pend, y_t):
                n, p0, pw, hbts, w_t = pend
                w2of = 2 * H
                for (s0, sw) in subs(pw):
                    o_p = po.tile([128, 512], f32, tag="o")
                    for hb in range(HB):
                        nc.tensor.matmul(
                            o_p[:, :sw],
                            w_t[:, w2of + hb * 128:w2of + (hb + 1) * 128],
                            hbts[hb][:, s0:s0 + sw],
                            start=(hb == 0), stop=(hb == HB - 1),
                        )
                    # copy into the per-head staging tile; single DMA at end
                    # of head. Split ACT/DVE to balance engine busy.
                    dst = y_t[:, p0 + s0:p0 + s0 + sw]
                    if copy_cnt[0] % copy_mod in act_copy:
                        nc.scalar.copy(dst, o_p[:, :sw])
                    else:
                        nc.vector.tensor_copy(dst, o_p[:, :sw])
                    copy_cnt[0] += 1

            in_tiles = {}

            def load(n):
                x_t = xp.tile([128, C], dmm, tag="x")
                nc.sync.dma_start(x_t[:], cast(xT[n]))
                w_t = wp.tile([128, WCOL], dmm, tag="w")
                nc.sync.dma_start(w_t[:], cast(wall[n]))
                in_tiles[n] = (x_t, w_t)

            def head(n):
                x_t, w_t = in_tiles.pop(n)
                y_t = ob.tile([128, C], din, tag="y")
                w1of, wgof, w2of = 0, H, 2 * H
                for (p0, pw) in pairs:
                    hbts = []
                    for hb in range(HB):
                        g_t = pg.tile([128, gW], f32, tag="g")
                        for (s0, sw) in subs(pw):
                            nc.tensor.matmul(
                                g_t[:, s0:s0 + sw],
                                w_t[:, wgof + hb * 128:wgof + (hb + 1) * 128],
                                x_t[:, p0 + s0:p0 + s0 + sw],
                                start=True, stop=True,
                            )
                        sil = sp.tile([128, gW], din, tag="sil")
                        nc.scalar.activation(sil[:, :pw], g_t[:, :pw], SILU)
                        hbt = hp.tile([128, gW], dmm, tag=f"hbt{hb}")
                        for (s0, sw) in subs(pw):
                            h1_t = ph.tile([128, 512], f32, tag="h1")
                            nc.tensor.matmul(
                                h1_t[:, :sw],
                                w_t[:, w1of + hb * 128:w1of + (hb + 1) * 128],
                                x_t[:, p0 + s0:p0 + s0 + sw],
                                start=True, stop=True,
                            )
                            nc.vector.tensor_mul(
                                hbt[:, s0:s0 + sw], h1_t[:, :sw],
                                sil[:, s0:s0 + sw]
                            )
                        hbts.append(hbt)
                        if pipe and hb == 0 and pending[0] is not None:
                            emit_o(*pending[0])
                            pending[0] = None
                    if pipe:
                        pending[0] = ((n, p0, pw, hbts, w_t), y_t)
                    else:
                        emit_o((n, p0, pw, hbts, w_t), y_t)
                return y_t

            def flush_and_dma(n, y_t):
                if out_dma == "sync":
                    nc.sync.dma_start(yT[n], y_t[:])
                else:
                    nc.scalar.dma_start(yT[n], y_t[:])

            def body():
                y_ts = {}
                load(0)
                load(1)
                for n in range(N):
                    if n + 2 < N:
                        load(n + 2)
                    # with pipe, head n-1's staging tile receives its last
                    # o-chunk during head n (deferred emit), so flush n-2;
                    # without pipe, n-1 is complete.
                    if pipe and n >= 2:
                        flush_and_dma(n - 2, y_ts.pop(n - 2))
                    elif not pipe and n >= 1:
                        flush_and_dma(n - 1, y_ts.pop(n - 1))
                    y_ts[n] = head(n)
                if pipe and pending[0] is not None:
                    emit_o(*pending[0])
                    pending[0] = None
                for n in sorted(y_ts):
                    flush_and_dma(n, y_ts.pop(n))

            if reps == 1:
                body()
            else:
                with tc.For_i(0, reps, 1):
                    body()
    nc.finalize()
    return nc


def _route(x, router_w):
    import jax
    import jax.numpy as jnp

    router_logits = jnp.asarray(x).reshape(B, N * D) @ jnp.asarray(router_w).T
    topk_logits, topk_idx = jax.lax.top_k(router_logits, 2)
    topk_w = jax.nn.softmax(topk_logits, axis=-1)
    return np.asarray(topk_idx), np.asarray(topk_w).astype(np.float32)


def _dispatch(x, topk_idx, topk_w):
    idx_list, wgt_list = [], []
    for e in range(E):
        sel = np.nonzero((topk_idx == e).any(axis=1))[0]
        we = np.where(topk_idx[sel, 0] == e, topk_w[sel, 0], topk_w[sel, 1])
        idx_list.append(sel)
        wgt_list.append(we.astype(np.float32))
    maxL = max(max(len(s) for s in idx_list), 1)
    C = ((maxL + 127) // 128) * 128
    chunks = []
    c0 = 0
    while c0 < C:
        cw = 512 if C - c0 >= 512 else C - c0
        chunks.append((c0, cw))
        c0 += cw
    return idx_list, wgt_list, C, tuple(chunks)


def _make_in_maps(x, w1, w_gate, w2, idx_list, wgt_list, C, bf16=USE_BF16):
    if bf16:
        import ml_dtypes
        dt = ml_dtypes.bfloat16
    else:
        dt = np.float32
    in_maps = []
    xTfull = np.ascontiguousarray(x.transpose(1, 2, 0).astype(dt))  # (N,128,B)
    for e in range(E):
        sel = idx_list[e]
        L = len(sel)
        xg = np.zeros((N, 128, C), dt)
        if L:
            xg[:, :, :L] = xTfull[:, :, sel]
        w2r = w2[e].reshape(N, HB, 128, 128).transpose(0, 2, 1, 3).reshape(N, 128, H)
        wcat = np.ascontiguousarray(np.concatenate(
            [w1[e].astype(dt), w_gate[e].astype(dt), w2r.astype(dt)], axis=2
        ))  # (N,128,3H)
        in_maps.append({"xT": xg, "wall": wcat})
    return in_maps


_runner_cache = {}


def _make_runner(nc):
    """Cached jitted executor equivalent to bass2jax.run_bass_via_pjrt,
    avoiding per-call retrace/rejit of the shard_map wrapper."""
    import jax
    import concourse.mybir as mybir
    from concourse import bass2jax
    from jax.sharding import Mesh, PartitionSpec
    from jax.experimental.shard_map import shard_map

    bass2jax.install_neuronx_cc_hook()
    partition_name = nc.partition_id_tensor.name if nc.partition_id_tensor else None
    in_names, out_names, out_avals, out_shapes = [], [], [], []
    for alloc in nc.m.functions[0].allocations:
        if not isinstance(alloc, mybir.MemoryLocationSet):
            continue
        name = alloc.memorylocations[0].name
        if alloc.kind == "ExternalInput":
            if name != partition_name:
                in_names.append(name)
        elif alloc.kind == "ExternalOutput":
            shape = tuple(alloc.tensor_shape)
            dtype = mybir.dt.np(alloc.dtype)
            out_names.append(name)
            out_avals.append(jax.core.ShapedArray(shape, dtype))
            out_shapes.append((shape, dtype))
    all_in_names = list(in_names) + list(out_names)
    if partition_name is not None:
        all_in_names.append(partition_name)

    def _body(*args):
        operands = list(args)
        if partition_name is not None:
            operands.append(bass2jax.partition_id_tensor())
        return tuple(bass2jax._bass_exec_p.bind(
            *operands,
            out_avals=tuple(out_avals),
            in_names=tuple(all_in_names),
            out_names=tuple(out_names),
            lowering_input_output_aliases=(),
            sim_require_finite=True,
            sim_require_nnan=True,
            nc=nc,
        ))

    mesh = Mesh(np.asarray(jax.devices()[:N_CORES]), ("core",))
    nio = len(in_names) + len(out_names)
    sharded = jax.jit(
        shard_map(_body, mesh=mesh,
                  in_specs=(PartitionSpec("core"),) * nio,
                  out_specs=(PartitionSpec("core"),) * len(out_names),
                  check_rep=False),
        keep_unused=True,
    )

    def run(in_maps):
        concat_in = [
            np.concatenate([np.asarray(in_maps[c][nm]) for c in range(N_CORES)],
                           axis=0)
            for nm in in_names
        ]
        concat_zeros = [
            np.zeros((N_CORES * s[0], *s[1:]), d) for (s, d) in out_shapes
        ]
        outs = sharded(*(concat_in + concat_zeros))
        outs = [np.asarray(o) for o in outs]
        results = []
        for c in range(N_CORES):
            res = {}
            for (nm, o, (s, d)) in zip(out_names, outs, out_shapes):
                res[nm] = o[c * s[0]:(c + 1) * s[0]]
            results.append(res)
        return results

    return run


def kernel(**inputs):
    x = np.asarray(inputs["x"], dtype=np.float32)
    router_w = np.asarray(inputs["router_w"], dtype=np.float32)
    w1 = np.asarray(inputs["w1"], dtype=np.float32)
    w_gate = np.asarray(inputs["w_gate"], dtype=np.float32)
    w2 = np.asarray(inputs["w2"], dtype=np.float32)

    topk_idx, topk_w = _route(x, router_w)
    idx_list, wgt_list, C, chunks = _dispatch(x, topk_idx, topk_w)

    key = (C, chunks, 1, USE_BF16)
    if key not in _nc_cache:
        _nc_cache[key] = _build_bass(C, chunks)
    nc = _nc_cache[key]

    in_maps = _make_in_maps(x, w1, w_gate, w2, idx_list, wgt_list, C)

    if key not in _runner_cache:
        from concourse import bass_utils
        res = bass_utils.run_bass_kernel_spmd(
            nc, in_maps, core_ids=list(range(N_CORES)), trace=False
        )
        results = res.results
        _runner_cache[key] = _make_runner(nc)
    else:
        results = _runner_cache[key](in_maps)

    out = np.zeros((B, N, D), np.float32)
    for e in range(E):
        sel = idx_list[e]
        L = len(sel)
        if L:
            yT = np.asarray(results[e]["yT"], dtype=np.float32)  # (N,128,C)
            out[sel] += yT[:, :, :L].transpose(2, 0, 1) * \
                wgt_list[e][:, None, None]
    return out



# revision 37
# speedup vs baseline: 1.1412x; 1.0046x over previous
import numpy as np

# Per-head sparse MoE (top-2 of 8 experts), expert-parallel across 8 NeuronCores.
# B=8192 tokens, N=16 heads, D=128, H=384, E=8.
# Host: router (replicates reference jnp ops bit-exactly) + token dispatch.
# Device (core e = expert e): per head, h1=w1.T@x, g=wg.T@x, h=h1*silu(g),
# y=w2.T@h on capacity-padded token batches; the per-token routing weight is
# applied on the HOST during unpack (it commutes through the w2 contraction
# along the token axis), so the device never touches it.
#
# Design notes (2026-08 remeasure): the kernel is jointly bound by PSUM
# evacuation on ACT+DVE (every fp32 PSUM column must pass through one of
# them exactly once: silu 48*C cols, mul 48*C, out-copy 16*C per core) and
# the PE matmul stream (144*C cycles; HW-measured floor ~118us). Measured
# per-op (rotating dests): DVE mul 512w PSUM*f32->bf16 627ns; mixed-dtype
# mul is SLOWER (800ns) so sil stays f32; copies ~635-660ns on either
# engine. Output copies go to a per-head staging tile (single DMA per head
# from the SP queue - DMA issue from ACT costs 667ns/op and was 40us of
# ACT time). PSUM: g ring2 [128,1024] (2 banks each), h1 ring3 [128,512],
# o ring1 = 8 banks. 1024-wide PSUM muls are 18%/col cheaper but need
# 2-bank h tiles - no PSUM layout fits ring>=2, and ring1 stalls PE.
# x/weights/y travel bf16; accumulate f32 in PSUM.

B, N, D, H, E = 8192, 16, 128, 384, 8
N_CORES = 8
HB = H // 128   # h-blocks of 128
WCOL = 3 * H    # packed weight columns per head: [w1 | wg | w2r]

USE_BF16 = True
PSUM_LAYOUT = "g2h3o1"   # g ring 2 (2 banks each), h1 ring 3, o ring 1

_nc_cache = {}


def _build_bass(C, chunks=None, reps=1, bf16=USE_BF16, layout=None,
                dve_res=None, sp_bufs=6, hp_bufs=6, pipe=True,
                act_copy=(0, 1, 2, 3), copy_mod=4, out_dma="sync",
                o_interleave=False, probe=None, hintl=0, sil_f32=True,
                hi_copy=False):
    import concourse.bacc as bacc
    import concourse.mybir as mybir
    import concourse.tile as tile

    layout = layout or PSUM_LAYOUT
    f32 = mybir.dt.float32
    f32r = mybir.dt.float32r
    bf = mybir.dt.bfloat16
    din = bf if bf16 else f32
    dmm = bf if bf16 else f32r
    SILU = mybir.ActivationFunctionType.Silu

    # layout: gW = g tile width, bg/bh/bo = PSUM ring depths; h tiles are
    # [128,512] (1 bank) unless layout starts with a capital H (then 1024).
    # oW: o tile width (1024 = 2-bank tile, halves copy count, ACT-only).
    cfg = {
        "g2h2o2": (1024, 2, 2, 2, 512, 512),
        "g2h3o1": (1024, 2, 3, 1, 512, 512),
        "g1H2o2": (1024, 1, 2, 2, 1024, 512),
        "G1h2o2": (2048, 1, 2, 2, 512, 512),
        "g2h2O1": (1024, 2, 2, 1, 512, 1024),
        "g2H1o2": (1024, 2, 1, 2, 1024, 512),
    }
    gW, bg, bh, bo, hW, oW = cfg[layout]

    pairs = [(p0, min(gW, C - p0)) for p0 in range(0, C, gW)]

    def subs(pw):
        return [(s, min(512, pw - s)) for s in range(0, pw, 512)]

    nc = bacc.Bacc("TRN2", target_bir_lowering=False, debug=False, num_devices=N_CORES)
    xT = nc.dram_tensor("xT", [N, 128, C], din, kind="ExternalInput").ap()
    wall = nc.dram_tensor("wall", [N, 128, WCOL], din, kind="ExternalInput").ap()
    yT = nc.dram_tensor("yT", [N, 128, C], din, kind="ExternalOutput").ap()

    def cast(ap):
        return ap if bf16 else ap.bitcast(f32r)

    copy_cnt = [0]

    with tile.TileContext(nc) as tc:
        with tc.tile_pool(name="xp", bufs=3) as xp, \
             tc.tile_pool(name="wp", bufs=3) as wp, \
             tc.tile_pool(name="sp", bufs=sp_bufs) as sp, \
             tc.tile_pool(name="hp", bufs=hp_bufs) as hp, \
             tc.tile_pool(name="ob", bufs=3) as ob, \
             tc.tile_pool(name="pg", bufs=bg, space="PSUM") as pg, \
             tc.tile_pool(name="ph", bufs=bh, space="PSUM") as ph, \
             tc.tile_pool(name="po", bufs=bo, space="PSUM") as po:

            pending = [None]
            dum = []

            def emit_copy(o_p, y_t, p0, s0, sw):
                if probe is not None:
                    return
                # copy into the per-head staging tile; single DMA at end
                # of head. Split ACT/DVE to balance engine busy.
                import contextlib
                dst = y_t[:, p0 + s0:p0 + s0 + sw]
                prio = tc.high_priority() if hi_copy else contextlib.nullcontext()
                with prio:
                    if copy_cnt[0] % copy_mod in act_copy:
                        nc.scalar.copy(dst, o_p[:, :sw])
                    else:
                        nc.vector.tensor_copy(dst, o_p[:, :sw])
                copy_cnt[0] += 1

            def emit_o(pend, y_t):
                n, p0, pw, hbts, w_t = pend
                w2of = 2 * H
                if o_interleave:
                    # hb-outer ordering: one stationary load per h-block,
                    # accumulating into all sub-chunk banks before switching.
                    o_ps = []
                    for _ in subs(pw):
                        o_sub = po.tile([128, 512], f32, tag="o")
                        o_ps.append(o_sub)
                    for hb in range(HB):
                        for i, (s0, sw) in enumerate(subs(pw)):
                            nc.tensor.matmul(
                                o_ps[i][:, :sw],
                                w_t[:, w2of + hb * 128:w2of + (hb + 1) * 128],
                                hbts[hb][:, s0:s0 + sw],
                                start=(hb == 0), stop=(hb == HB - 1),
                            )
                    for i, (s0, sw) in enumerate(subs(pw)):
                        emit_copy(o_ps[i], y_t, p0, s0, sw)
                    return
                if oW == 1024:
                    # o tile spans 2 banks; each MM writes one 512 half
                    # (single-bank), then ONE wide ACT copy evacuates both.
                    for (q0, qw) in [(s, min(1024, pw - s))
                                     for s in range(0, pw, 1024)]:
                        o_p = po.tile([128, 1024], f32, tag="o")
                        for (s0, sw) in [(s, min(512, qw - s))
                                         for s in range(0, qw, 512)]:
                            for hb in range(HB):
                                nc.tensor.matmul(
                                    o_p[:, s0:s0 + sw],
                                    w_t[:, w2of + hb * 128:
                                         w2of + (hb + 1) * 128],
                                    hbts[hb][:, q0 + s0:q0 + s0 + sw],
                                    start=(hb == 0), stop=(hb == HB - 1),
                                )
                        if probe is None:
                            nc.scalar.copy(
                                y_t[:, p0 + q0:p0 + q0 + qw], o_p[:, :qw])
                    return
                for (s0, sw) in subs(pw):
                    o_p = po.tile([128, 512], f32, tag="o")
                    for hb in range(HB):
                        nc.tensor.matmul(
                            o_p[:, :sw],
                            w_t[:, w2of + hb * 128:w2of + (hb + 1) * 128],
                            hbts[hb][:, s0:s0 + sw],
                            start=(hb == 0), stop=(hb == HB - 1),
                        )
                    emit_copy(o_p, y_t, p0, s0, sw)

            in_tiles = {}

            def load(n):
                x_t = xp.tile([128, C], dmm, tag="x")
                nc.sync.dma_start(x_t[:], cast(xT[n]))
                w_t = wp.tile([128, WCOL], dmm, tag="w")
                nc.sync.dma_start(w_t[:], cast(wall[n]))
                in_tiles[n] = (x_t, w_t)

            def head(n):
                x_t, w_t = in_tiles.pop(n)
                y_t = ob.tile([128, C], din, tag="y")
                w1of, wgof, w2of = 0, H, 2 * H
                for (p0, pw) in pairs:
                    hbts = []
                    for hb in range(HB):
                        g_t = pg.tile([128, gW], f32, tag="g")
                        for (s0, sw) in subs(pw):
                            nc.tensor.matmul(
                                g_t[:, s0:s0 + sw],
                                w_t[:, wgof + hb * 128:wgof + (hb + 1) * 128],
                                x_t[:, p0 + s0:p0 + s0 + sw],
                                start=True, stop=True,
                            )
                        if probe in ("pe", "pemul"):
                            # engine-isolation probes: no silu (ACT idle);
                            # "pe" also drops the muls (DVE idle).
                            hbt_m = None
                            if probe == "pemul":
                                hbt_m = hp.tile([128, gW], dmm,
                                                tag=f"hbt{hb}")
                            for (s0, sw) in subs(pw):
                                h1_t = ph.tile([128, 512], f32, tag="h1")
                                nc.tensor.matmul(
                                    h1_t[:, :sw],
                                    w_t[:, w1of + hb * 128:w1of + (hb + 1) * 128],
                                    x_t[:, p0 + s0:p0 + s0 + sw],
                                    start=True, stop=True,
                                )
                                if hbt_m is not None:
                                    nc.vector.tensor_mul(
                                        hbt_m[:, s0:s0 + sw], h1_t[:, :sw],
                                        dum[hb][:, s0:s0 + sw])
                            hbts.append(hbt_m if hbt_m is not None
                                        else dum[hb])
                            continue
                        sil = sp.tile([128, gW], f32 if sil_f32 else din, tag="sil")
                        nc.scalar.activation(sil[:, :pw], g_t[:, :pw], SILU)
                        if probe == "pesilu":
                            # PE + ACT silu; no muls/copies (DVE idle).
                            for (s0, sw) in subs(pw):
                                h1_t = ph.tile([128, 512], f32, tag="h1")
                                nc.tensor.matmul(
                                    h1_t[:, :sw],
                                    w_t[:, w1of + hb * 128:w1of + (hb + 1) * 128],
                                    x_t[:, p0 + s0:p0 + s0 + sw],
                                    start=True, stop=True,
                                )
                            hbts.append(dum[hb])
                            continue
                        hbt = hp.tile([128, gW], dmm, tag=f"hbt{hb}")
                        if hW == 512:
                            for (s0, sw) in subs(pw):
                                h1_t = ph.tile([128, 512], f32, tag="h1")
                                nc.tensor.matmul(
                                    h1_t[:, :sw],
                                    w_t[:, w1of + hb * 128:w1of + (hb + 1) * 128],
                                    x_t[:, p0 + s0:p0 + s0 + sw],
                                    start=True, stop=True,
                                )
                                nc.vector.tensor_mul(
                                    hbt[:, s0:s0 + sw], h1_t[:, :sw],
                                    sil[:, s0:s0 + sw]
                                )
                        else:
                            h1_t = ph.tile([128, hW], f32, tag="h1")
                            for (s0, sw) in subs(pw):
                                nc.tensor.matmul(
                                    h1_t[:, s0:s0 + sw],
                                    w_t[:, w1of + hb * 128:w1of + (hb + 1) * 128],
                                    x_t[:, p0 + s0:p0 + s0 + sw],
                                    start=True, stop=True,
                                )
                            nc.vector.tensor_mul(
                                hbt[:, :pw], h1_t[:, :pw], sil[:, :pw]
                            )
                        hbts.append(hbt)
                        if pipe and hb == 0 and pending[0] is not None:
                            emit_o(*pending[0])
                            pending[0] = None
                    if pipe:
                        pending[0] = ((n, p0, pw, hbts, w_t), y_t)
                    else:
                        emit_o((n, p0, pw, hbts, w_t), y_t)
                return y_t

            def flush_and_dma(n, y_t):
                if probe is not None:
                    return
                if out_dma == "sync":
                    nc.sync.dma_start(yT[n], y_t[:])
                else:
                    nc.scalar.dma_start(yT[n], y_t[:])

            def head_gen(n):
                """Per-pair generator variant of head() for interleaving.
                No pipe: emit_o inline."""
                x_t, w_t = in_tiles.pop(n)
                y_t = ob.tile([128, C], din, tag="y")
                w1of, wgof = 0, H
                for (p0, pw) in pairs:
                    hbts = []
                    for hb in range(HB):
                        g_t = pg.tile([128, gW], f32, tag="g")
                        for (s0, sw) in subs(pw):
                            nc.tensor.matmul(
                                g_t[:, s0:s0 + sw],
                                w_t[:, wgof + hb * 128:wgof + (hb + 1) * 128],
                                x_t[:, p0 + s0:p0 + s0 + sw],
                                start=True, stop=True,
                            )
                        sil = sp.tile([128, gW], f32 if sil_f32 else din, tag="sil")
                        nc.scalar.activation(sil[:, :pw], g_t[:, :pw], SILU)
                        hbt = hp.tile([128, gW], dmm, tag=f"hbt{hb}")
                        for (s0, sw) in subs(pw):
                            h1_t = ph.tile([128, 512], f32, tag="h1")
                            nc.tensor.matmul(
                                h1_t[:, :sw],
                                w_t[:, w1of + hb * 128:w1of + (hb + 1) * 128],
                                x_t[:, p0 + s0:p0 + s0 + sw],
                                start=True, stop=True,
                            )
                            nc.vector.tensor_mul(
                                hbt[:, s0:s0 + sw], h1_t[:, :sw],
                                sil[:, s0:s0 + sw]
                            )
                        hbts.append(hbt)
                        yield
                    emit_o((n, p0, pw, hbts, w_t), y_t)
                    yield
                flush_and_dma(n, y_t)

            def body_interleaved():
                k = hintl
                for base in range(0, N, k):
                    grp = list(range(base, min(base + k, N)))
                    for n in grp:
                        if n + k < N and (n + k) not in in_tiles:
                            load(n + k)
                    gens = [head_gen(n) for n in grp]
                    alive = list(gens)
                    while alive:
                        for g in list(alive):
                            try:
                                next(g)
                            except StopIteration:
                                alive.remove(g)

            def body():
                y_ts = {}
                if probe is not None:
                    for hb in range(HB):
                        d_t = hp.tile([128, gW], dmm, tag=f"dum{hb}")
                        nc.vector.memset(d_t[:], 0.25)
                        dum.append(d_t)
                load(0)
                load(1)
                if hintl:
                    for n in range(2, min(hintl, N)):
                        load(n)
                    body_interleaved()
                    return
                for n in range(N):
                    if n + 2 < N:
                        load(n + 2)
                    # with pipe, head n-1's staging tile receives its last
                    # o-chunk during head n (deferred emit), so flush n-2;
                    # without pipe, n-1 is complete.
                    if pipe and n >= 2:
                        flush_and_dma(n - 2, y_ts.pop(n - 2))
                    elif not pipe and n >= 1:
                        flush_and_dma(n - 1, y_ts.pop(n - 1))
                    y_ts[n] = head(n)
                if pipe and pending[0] is not None:
                    emit_o(*pending[0])
                    pending[0] = None
                for n in sorted(y_ts):
                    flush_and_dma(n, y_ts.pop(n))

            if reps == 1:
                body()
            else:
                with tc.For_i(0, reps, 1):
                    body()
    nc.finalize()
    return nc


def _route(x, router_w):
    import jax
    import jax.numpy as jnp

    router_logits = jnp.asarray(x).reshape(B, N * D) @ jnp.asarray(router_w).T
    topk_logits, topk_idx = jax.lax.top_k(router_logits, 2)
    topk_w = jax.nn.softmax(topk_logits, axis=-1)
    return np.asarray(topk_idx), np.asarray(topk_w).astype(np.float32)


def _dispatch(x, topk_idx, topk_w):
    idx_list, wgt_list = [], []
    for e in range(E):
        sel = np.nonzero((topk_idx == e).any(axis=1))[0]
        we = np.where(topk_idx[sel, 0] == e, topk_w[sel, 0], topk_w[sel, 1])
        idx_list.append(sel)
        wgt_list.append(we.astype(np.float32))
    maxL = max(max(len(s) for s in idx_list), 1)
    C = ((maxL + 127) // 128) * 128
    chunks = []
    c0 = 0
    while c0 < C:
        cw = 512 if C - c0 >= 512 else C - c0
        chunks.append((c0, cw))
        c0 += cw
    return idx_list, wgt_list, C, tuple(chunks)


def _make_in_maps(x, w1, w_gate, w2, idx_list, wgt_list, C, bf16=USE_BF16):
    if bf16:
        import ml_dtypes
        dt = ml_dtypes.bfloat16
    else:
        dt = np.float32
    in_maps = []
    xTfull = np.ascontiguousarray(x.transpose(1, 2, 0).astype(dt))  # (N,128,B)
    for e in range(E):
        sel = idx_list[e]
        L = len(sel)
        xg = np.zeros((N, 128, C), dt)
        if L:
            xg[:, :, :L] = xTfull[:, :, sel]
        w2r = w2[e].reshape(N, HB, 128, 128).transpose(0, 2, 1, 3).reshape(N, 128, H)
        wcat = np.ascontiguousarray(np.concatenate(
            [w1[e].astype(dt), w_gate[e].astype(dt), w2r.astype(dt)], axis=2
        ))  # (N,128,3H)
        in_maps.append({"xT": xg, "wall": wcat})
    return in_maps


_runner_cache = {}


def _make_runner(nc):
    """Cached jitted executor equivalent to bass2jax.run_bass_via_pjrt,
    avoiding per-call retrace/rejit of the shard_map wrapper."""
    import jax
    import concourse.mybir as mybir
    from concourse import bass2jax
    from jax.sharding import Mesh, PartitionSpec
    from jax.experimental.shard_map import shard_map

    bass2jax.install_neuronx_cc_hook()
    partition_name = nc.partition_id_tensor.name if nc.partition_id_tensor else None
    in_names, out_names, out_avals, out_shapes = [], [], [], []
    for alloc in nc.m.functions[0].allocations:
        if not isinstance(alloc, mybir.MemoryLocationSet):
            continue
        name = alloc.memorylocations[0].name
        if alloc.kind == "ExternalInput":
            if name != partition_name:
                in_names.append(name)
        elif alloc.kind == "ExternalOutput":
            shape = tuple(alloc.tensor_shape)
            dtype = mybir.dt.np(alloc.dtype)
            out_names.append(name)
            out_avals.append(jax.core.ShapedArray(shape, dtype))
            out_shapes.append((shape, dtype))
    all_in_names = list(in_names) + list(out_names)
    if partition_name is not None:
        all_in_names.append(partition_name)

    def _body(*args):
        operands = list(args)
        if partition_name is not None:
            operands.append(bass2jax.partition_id_tensor())
        return tuple(bass2jax._bass_exec_p.bind(
            *operands,
            out_avals=tuple(out_avals),
            in_names=tuple(all_in_names),
            out_names=tuple(out_names),
            lowering_input_output_aliases=(),
            sim_require_finite=True,
            sim_require_nnan=True,
            nc=nc,
        ))

    mesh = Mesh(np.asarray(jax.devices()[:N_CORES]), ("core",))
    nio = len(in_names) + len(out_names)
    sharded = jax.jit(
        shard_map(_body, mesh=mesh,
                  in_specs=(PartitionSpec("core"),) * nio,
                  out_specs=(PartitionSpec("core"),) * len(out_names),
                  check_rep=False),
        keep_unused=True,
    )

    def run(in_maps):
        concat_in = [
            np.concatenate([np.asarray(in_maps[c][nm]) for c in range(N_CORES)],
                           axis=0)
            for nm in in_names
        ]
        concat_zeros = [
            np.zeros((N_CORES * s[0], *s[1:]), d) for (s, d) in out_shapes
        ]
        outs = sharded(*(concat_in + concat_zeros))
        outs = [np.asarray(o) for o in outs]
        results = []
        for c in range(N_CORES):
            res = {}
            for (nm, o, (s, d)) in zip(out_names, outs, out_shapes):
                res[nm] = o[c * s[0]:(c + 1) * s[0]]
            results.append(res)
        return results

    return run


def kernel(**inputs):
    x = np.asarray(inputs["x"], dtype=np.float32)
    router_w = np.asarray(inputs["router_w"], dtype=np.float32)
    w1 = np.asarray(inputs["w1"], dtype=np.float32)
    w_gate = np.asarray(inputs["w_gate"], dtype=np.float32)
    w2 = np.asarray(inputs["w2"], dtype=np.float32)

    topk_idx, topk_w = _route(x, router_w)
    idx_list, wgt_list, C, chunks = _dispatch(x, topk_idx, topk_w)

    key = (C, chunks, 1, USE_BF16)
    if key not in _nc_cache:
        _nc_cache[key] = _build_bass(C, chunks)
    nc = _nc_cache[key]

    in_maps = _make_in_maps(x, w1, w_gate, w2, idx_list, wgt_list, C)

    if key not in _runner_cache:
        from concourse import bass_utils
        res = bass_utils.run_bass_kernel_spmd(
            nc, in_maps, core_ids=list(range(N_CORES)), trace=False
        )
        results = res.results
        _runner_cache[key] = _make_runner(nc)
    else:
        results = _runner_cache[key](in_maps)

    out = np.zeros((B, N, D), np.float32)
    for e in range(E):
        sel = idx_list[e]
        L = len(sel)
        if L:
            yT = np.asarray(results[e]["yT"], dtype=np.float32)  # (N,128,C)
            out[sel] += yT[:, :, :L].transpose(2, 0, 1) * \
                wgt_list[e][:, None, None]
    return out

